# revision 3
# baseline (speedup 1.0000x reference)
"""Trainium2 Bass kernel v2 for nn_DetectionLoss — PE outer-product pair stage.

Key ideas vs baseline:
  - IoU pair stage via PE rank-1 matmuls: anchors separate into per-(w,a)
    x-tables and per-(h,a) y-tables; inter[h,w] = wy[h] (x) wx[w] per (a,m).
    wx is pre-scaled by 1/(areaA+areaB) so the packed key is monotone in
    u = inter/c and the per-pair union/divide disappear (iou >= .5 <=> u >= 1/3).
  - Packed key [u25|31-m|label2] max-reduced over m with a strided X-reduce.
  - fp16 select loop (ts is_equal*payload + tt add) and fp16 zt for the
    4x-mode binary-search count passes.
  - partition_all_reduce instead of slow C-reduces; 12-iter binary search,
    both images merged into one [128,2] chain.
  - masked sums fused via accum_out; softplus via the Softplus act table.

Program order (software pipeline): P0 P1 M0 M1 K SEL0 CL0 SEL1 CL1 SRCH FIN.
"""
import os as _os

import numpy as np

import concourse.bass as bass
import concourse.bacc as bacc
import concourse.bass_isa as bass_isa
import concourse.mybir as mybir
from concourse.tile import TileContext
from concourse.bass_utils import run_bass_kernel_spmd

F32 = mybir.dt.float32
F16 = mybir.dt.float16
U32 = mybir.dt.uint32
I32 = mybir.dt.int32
OP = mybir.AluOpType
AF = mybir.ActivationFunctionType
AX = mybir.AxisListType
RED = bass_isa.ReduceOp

B, C, A, H, W, M = 16, 3, 9, 128, 128, 32
K = 5 + C
N_CORES = 8
N_IMG = B // N_CORES
AW = A * W

PK_MASK = 0xFFFFFF80
IDX_MASK = 0x7C
TGT_MASK = 0x3
POS_TH = float(np.float32(1.0 / 3.0))
NEG_TH = float(np.float32(3.0 / 13.0))
ZBIG = 32768.0
N_SEARCH = int(_os.environ.get("KK_NSEARCH", "10"))
SEARCH_SPAN = 8.0
STOP_STAGE = int(_os.environ.get("KK_STOP_STAGE", "99"))


def _stt_u32imm(eng, out, in0, imm, in1, op0, op1):
    return eng.add_instruction(
        mybir.InstTensorScalarPtr(
            name=eng.bass.get_next_instruction_name(),
            is_scalar_tensor_tensor=True,
            op0=op0, op1=op1,
            ins=[eng.lower_ap(in0),
                 mybir.ImmediateValue(dtype=U32, value=imm),
                 eng.lower_ap(in1)],
            outs=[eng.lower_ap(out)],
        ))


def _ts_u32imm(eng, out, in_, imm, op):
    return eng.add_instruction(
        mybir.InstTensorScalarPtr(
            name=eng.bass.get_next_instruction_name(),
            op0=op, op1=OP.bypass,
            ins=[eng.lower_ap(in_),
                 mybir.ImmediateValue(dtype=U32, value=imm)],
            outs=[eng.lower_ap(out)],
        ))


def _ts2_u32imm(eng, out, in_, imm1, op0, imm2, op1):
    return eng.add_instruction(
        mybir.InstTensorScalarPtr(
            name=eng.bass.get_next_instruction_name(),
            op0=op0, op1=op1,
            ins=[eng.lower_ap(in_),
                 mybir.ImmediateValue(dtype=U32, value=imm1),
                 mybir.ImmediateValue(dtype=U32, value=imm2)],
            outs=[eng.lower_ap(out)],
        ))


def build_nc(n_img=N_IMG):
    nc = bacc.Bacc("TRN2", target_bir_lowering=False, debug=False)
    d_pred = nc.declare_dram_parameter("pred", [n_img, A * K, H, W], F32, isOutput=False)
    d_bd = nc.declare_dram_parameter("bd", [n_img, A, 2, 16, 2048], F16, isOutput=False)
    d_wyt = nc.declare_dram_parameter("wyt", [n_img, A, 2, 16, 128], F16, isOutput=False)
    d_gtl = nc.declare_dram_parameter("gtl", [n_img, H, M], F32, isOutput=False)
    d_ipat = nc.declare_dram_parameter("ipat", [H, M], U32, isOutput=False)
    d_pb = nc.declare_dram_parameter("pb", [n_img, H, 2 * M], U32, isOutput=False)
    d_loc = nc.declare_dram_parameter("locp", [6, H, AW], F32, isOutput=False)
    d_res = nc.declare_dram_parameter("res", [1, 8], F32, isOutput=True)

    V, G, S = nc.vector, nc.gpsimd, nc.scalar

    def finish_early(tag):
        res_t = nc.vector  # noqa: placeholder (unused)

    with TileContext(nc) as tc:
        with (
            tc.tile_pool(name="pers", bufs=1) as pe,
            tc.tile_pool(name="img", bufs=2) as pi,
            tc.tile_pool(name="pred_p", bufs=1) as ppd,
            tc.tile_pool(name="pair", bufs=2) as pr,
            tc.tile_pool(name="flat", bufs=2) as pf,
            tc.tile_pool(name="scr", bufs=1) as ps,
            tc.tile_pool(name="sel", bufs=1) as pl,
            tc.tile_pool(name="tiny", bufs=1) as pt,
            tc.tile_pool(name="psum", bufs=2, space="PSUM") as pp,
        ):
            # ---------------- persistent loads ----------------
            locp = [pe.tile([H, AW], F32, name=f"locp{i}") for i in range(2)]
            for i in range(2):
                nc.sync.dma_start(out=locp[i][:, :], in_=d_loc[i])
            invaw, invah = locp
            logaw = logah = None

            ipat = pe.tile([H, M], U32, name="ipat")
            nc.sync.dma_start(out=ipat[:, :], in_=d_ipat[:, :])
            negone = pe.tile([128, 1], F32, name="negone")
            V.memset(negone[:, :], -1.0)

            st = [dict() for _ in range(n_img)]

            # ============ phase P: pair stage per image ============
            for i in range(n_img):
                lab = pi.tile([H, M], F32, name=f"lab{i}", tag="lab")
                nc.sync.dma_start(out=lab[:, :], in_=d_gtl[i])

                lm1 = pt.tile([H, M], F32, name=f"lm1_{i}", tag="lm1")
                V.tensor_scalar(out=lm1[:, :], in0=lab[:, :], scalar1=1.0,
                                scalar2=0.0, op0=OP.subtract, op1=OP.max)
                V.tensor_single_scalar(out=lm1[:, :], in_=lm1[:, :],
                                       scalar=float(C - 1), op=OP.min)
                lm1u = pt.tile([H, M], U32, name=f"lm1u_{i}", tag="lm1u")
                V.tensor_copy(out=lm1u[:, :], in_=lm1[:, :])
                ipf = pi.tile([H, M], U32, name=f"ipf{i}", tag="ipf")
                V.tensor_tensor(out=ipf[:, :], in0=ipat[:, :], in1=lm1u[:, :],
                                op=OP.bitwise_or)

                pmax = pi.tile([H, AW], F32, name=f"pmax{i}", tag="pmax")
                for a in range(A):
                    for half in range(2):
                        bd = pf.tile([16, 2048], F16, name=f"bd{i}_{a}_{half}",
                                     tag="bd")
                        wyt = pf.tile([16, 128], F16, name=f"wyt{i}_{a}_{half}",
                                      tag="wyt")
                        nc.sync.dma_start(out=bd[:, :], in_=d_bd[i, a, half])
                        nc.sync.dma_start(out=wyt[:, :], in_=d_wyt[i, a, half])
                        psm = pp.tile([128, 2048], F32, name=f"ps{i}_{a}_{half}",
                                      tag="interp")
                        for nblk in range(4):
                            nc.tensor.matmul(psm[:, nblk * 512:(nblk + 1) * 512],
                                             wyt[:, :],
                                             bd[:, nblk * 512:(nblk + 1) * 512],
                                             start=True, stop=True)
                        _stt_u32imm(
                            V, psm[:, :].bitcast(U32), psm[:, :].bitcast(U32),
                            PK_MASK,
                            ipf[:, half * 16:(half + 1) * 16]
                            .unsqueeze(2).broadcast_to([128, 16, 128]),
                            OP.bitwise_and, OP.bitwise_or)
                        pkv = psm.rearrange("p (m w) -> p w m", m=16)
                        if half == 0:
                            V.tensor_reduce(out=pmax[:, a * 128:(a + 1) * 128],
                                            in_=pkv, axis=AX.X, op=OP.max)
                        else:
                            red = pr.tile([128, 128], F32, name=f"red_{i}_{a}",
                                          tag="red")
                            V.tensor_reduce(out=red[:, :], in_=pkv, axis=AX.X,
                                            op=OP.max)
                            V.tensor_tensor(out=pmax[:, a * 128:(a + 1) * 128],
                                            in0=pmax[:, a * 128:(a + 1) * 128],
                                            in1=red[:, :], op=OP.max)
                st[i]["pmax"] = pmax

            def early_out():
                res_t = pt.tile([1, 8], F32, name="res_t0", tag="res_t")
                V.memset(res_t[:, :], 0.0)
                nc.sync.dma_start(out=d_res[:, :], in_=res_t[:, :])

            if STOP_STAGE < 2:
                early_out()
                nc.compile()
                return nc

            # counts: [npos0, npos1, nneg0, nneg1]
            cnt4 = pe.tile([128, 4], F32, name="cnt4")

            # ============ phase M: masks from pmax (no pred needed) ============
            for i in range(n_img):
                pmax = st[i]["pmax"]
                pmu = pmax.bitcast(U32)
                bq = ps.tile([H, AW], F32, name=f"bq{i}", tag="spA")
                _ts_u32imm(V, bq[:, :].bitcast(U32), pmu[:, :], PK_MASK, OP.bitwise_and)
                pos_f = pi.tile([H, AW], F32, name=f"pos{i}", tag="pos")
                V.tensor_scalar(out=pos_f[:, :], in0=bq[:, :], scalar1=POS_TH,
                                scalar2=None, op0=OP.is_ge, op1=OP.add,
                                accum_out=cnt4[:, i:i + 1])
                neg_f = pi.tile([H, AW], F32, name=f"neg{i}", tag="neg")
                V.tensor_scalar(out=neg_f[:, :], in0=bq[:, :], scalar1=NEG_TH,
                                scalar2=None, op0=OP.is_lt, op1=OP.add,
                                accum_out=cnt4[:, 2 + i:3 + i])
                st[i]["pos"] = pos_f
                st[i]["neg"] = neg_f

            if STOP_STAGE < 3:
                early_out()
                nc.compile()
                return nc

            # ---- k computation on [128,2] (both images) ----
            cnt4a = pe.tile([128, 4], F32, name="cnt4a")
            G.partition_all_reduce(cnt4a[:, :], cnt4[:, :], channels=128,
                                   reduce_op=RED.add)
            npos2 = cnt4a[:, 0:2]
            nneg2 = cnt4a[:, 2:4]

            def t2(nm):
                return pt.tile([128, 2], F32, name=nm, tag=nm)

            np3 = t2("np3")
            V.tensor_single_scalar(out=np3[:, :], in_=npos2, scalar=3.0, op=OP.mult)
            kmin = t2("kmin")
            V.tensor_tensor(out=kmin[:, :], in0=np3[:, :], in1=nneg2, op=OP.min)
            nn10 = t2("nn10")
            V.tensor_single_scalar(out=nn10[:, :], in_=nneg2, scalar=0.1, op=OP.mult)
            nn10i = pt.tile([128, 2], I32, name="nn10i", tag="nn10i")
            V.tensor_copy(out=nn10i[:, :], in_=nn10[:, :])
            nn10f = t2("nn10f")
            V.tensor_copy(out=nn10f[:, :], in_=nn10i[:, :])
            k2 = t2("k2")
            V.tensor_single_scalar(out=k2[:, :], in_=nn10f[:, :], scalar=1.0, op=OP.max)
            znn = t2("znn")
            V.tensor_single_scalar(out=znn[:, :], in_=nneg2, scalar=0.0, op=OP.is_gt)
            k2z = t2("k2z")
            V.tensor_tensor(out=k2z[:, :], in0=k2[:, :], in1=znn[:, :], op=OP.mult)
            zf = t2("zf")
            V.tensor_single_scalar(out=zf[:, :], in_=npos2, scalar=0.0, op=OP.is_equal)
            kd = t2("kd")
            V.tensor_tensor(out=kd[:, :], in0=k2z[:, :], in1=kmin[:, :], op=OP.subtract)
            kzd = t2("kzd")
            V.tensor_tensor(out=kzd[:, :], in0=zf[:, :], in1=kd[:, :], op=OP.mult)
            kk = pe.tile([128, 2], F32, name="kk")
            V.tensor_tensor(out=kk[:, :], in0=kmin[:, :], in1=kzd[:, :], op=OP.add)

            # masked-sum columns: [cls0, cls1, loc0, loc1, obj0, obj1]
            fin6 = pe.tile([128, 6], F32, name="fin6")

            # ============ per image: SEL then CL ============
            for i in range(n_img):
                pmax = st[i]["pmax"]
                pos_f = st[i]["pos"]
                neg_f = st[i]["neg"]

                # ---- SEL: paired-payload select loop (u32 mask/or) ----
                pb = pt.tile([H, 2 * M], U32, name=f"pb{i}", tag="pb")
                nc.sync.dma_start(out=pb[:, :], in_=d_pb[i])
                pb2 = pb.rearrange("p (m two) -> p m two", two=2)
                idq = ps.tile([H, AW], U32, name=f"idq{i}", tag="junkM")
                _ts_u32imm(V, idq[:, :], pmax.bitcast(U32)[:, :], IDX_MASK,
                           OP.bitwise_and)
                accb = pl.tile([H, 2, AW], U32, name=f"accb_{i}", tag="accb")
                V.memset(accb[:, :, :], 0)
                for m in range(M):
                    pattb = (31 - m) << 2
                    msk = ps.tile([H, AW], mybir.dt.uint8, name=f"msk{i}_{m}",
                                  tag="msk")
                    _ts_u32imm(V, msk[:, :], idq[:, :], pattb, OP.is_equal)
                    V.copy_predicated(out=accb[:, 0, :], mask=msk[:, :],
                                      data=pb2[:, m, 0:1].broadcast_to([H, AW]))
                    V.copy_predicated(out=accb[:, 1, :], mask=msk[:, :],
                                      data=pb2[:, m, 1:2].broadcast_to([H, AW]))
                accs = {"xy": accb[:, 0, :], "wh": accb[:, 1, :]}

                # ---- CL: pred-dependent stages ----
                pred_t = ppd.tile([H, K * A * W], F32, name=f"pred{i}", tag="pred")
                nc.sync.dma_start(out=pred_t[:, :], in_=d_pred[i].transpose([1, 0, 2]))
                pv = pred_t.rearrange("p (a k w) -> p a k w", a=A, k=K)
                pobj = pv[:, :, 4, :]

                # obj + mining inputs
                axp = ps.tile([H, AW], F32, name=f"axp{i}", tag="sAbs")
                S.activation(out=axp[:, :], in_=pobj, func=AF.Abs)
                exn = ps.tile([H, AW], F32, name=f"exn{i}", tag="sE0")
                S.activation(out=exn[:, :], in_=axp[:, :], func=AF.Exp, scale=-1.0)
                lgp = ps.tile([H, AW], F32, name=f"lgp{i}", tag="sE1")
                S.activation(out=lgp[:, :], in_=exn[:, :], func=AF.Ln, bias=1.0)
                rlp = ps.tile([H, AW], F32, name=f"rlp{i}", tag="sSq")
                S.activation(out=rlp[:, :], in_=pobj, func=AF.Relu)
                sp = ps.tile([H, AW], F32, name=f"sp{i}", tag="spF")
                V.tensor_tensor(out=sp[:, :], in0=rlp[:, :], in1=lgp[:, :], op=OP.add)
                spn = ps.tile([H, AW], F32, name=f"spn{i}", tag="spC")
                V.scalar_tensor_tensor(out=spn[:, :], in0=pobj, scalar=-1.0,
                                       in1=sp[:, :], op0=OP.mult, op1=OP.add)
                V.scalar_tensor_tensor(out=spn[:, :], in0=spn[:, :], scalar=1.0,
                                       in1=pos_f[:, :], op0=OP.mult, op1=OP.mult,
                                       accum_out=fin6[:, 4 + i:5 + i])
                spz = pi.tile([H, AW], F16, name=f"spz{i}", tag="spz")
                V.tensor_tensor(out=spz[:, :], in0=sp[:, :], in1=neg_f[:, :], op=OP.mult)
                st[i]["spz"] = spz
                zb = ps.tile([H, AW], F32, name=f"zb{i}", tag="spA")
                S.activation(out=zb[:, :], in_=pobj, func=AF.Copy, bias=ZBIG)
                zm = ps.tile([H, AW], F32, name=f"zm{i}", tag="spC")
                V.scalar_tensor_tensor(out=zm[:, :], in0=zb[:, :], scalar=0.0,
                                       in1=neg_f[:, :], op0=OP.add, op1=OP.mult)
                zt = pi.tile([H, AW], F16, name=f"zt{i}", tag="zt")
                V.tensor_single_scalar(out=zt[:, :], in_=zm[:, :], scalar=ZBIG,
                                       op=OP.subtract)
                st[i]["zt"] = zt

                # cls
                pc = [pv[:, :, 5 + c, :] for c in range(C)]
                ex0 = ps.tile([H, AW], F32, name=f"ex0{i}", tag="sE0")
                S.activation(out=ex0[:, :], in_=pc[0], func=AF.Exp)
                ex1 = ps.tile([H, AW], F32, name=f"ex1{i}", tag="sE1")
                S.activation(out=ex1[:, :], in_=pc[1], func=AF.Exp)
                es01 = ps.tile([H, AW], F32, name=f"es01{i}", tag="spD")
                V.tensor_tensor(out=es01[:, :], in0=ex0[:, :], in1=ex1[:, :], op=OP.add)
                ex2 = ps.tile([H, AW], F32, name=f"ex2{i}", tag="sE0")
                S.activation(out=ex2[:, :], in_=pc[2], func=AF.Exp)
                es = ps.tile([H, AW], F32, name=f"es{i}", tag="spC")
                V.tensor_tensor(out=es[:, :], in0=es01[:, :], in1=ex2[:, :], op=OP.add)
                lse = ps.tile([H, AW], F32, name=f"lse{i}", tag="sLse")
                S.activation(out=lse[:, :], in_=es[:, :], func=AF.Ln)

                tgu = ps.tile([H, AW], U32, name=f"tgu{i}", tag="junkM")
                _ts_u32imm(V, tgu[:, :], pmax.bitcast(U32)[:, :], TGT_MASK,
                           OP.bitwise_and)
                tgtf = ps.tile([H, AW], F32, name=f"tgtf{i}", tag="spA")
                V.tensor_copy(out=tgtf[:, :], in_=tgu[:, :])
                clsc = pt.tile([128, 4], F32, name=f"clsc{i}", tag="clsc")
                V.scalar_tensor_tensor(out=lse[:, :], in0=lse[:, :], scalar=1.0,
                                       in1=pos_f[:, :], op0=OP.mult, op1=OP.mult,
                                       accum_out=clsc[:, 0:1])
                for cix in range(C):
                    mc = ps.tile([H, AW], F32, name=f"mc{cix}_{i}", tag="spC")
                    V.tensor_single_scalar(out=mc[:, :], in_=tgtf[:, :],
                                           scalar=float(cix), op=OP.is_equal)
                    mp = ps.tile([H, AW], F32, name=f"mp{cix}_{i}", tag="spE")
                    V.tensor_tensor(out=mp[:, :], in0=mc[:, :], in1=pos_f[:, :],
                                    op=OP.mult)
                    V.scalar_tensor_tensor(out=mp[:, :], in0=pc[cix], scalar=1.0,
                                           in1=mp[:, :], op0=OP.mult, op1=OP.mult,
                                           accum_out=clsc[:, 1 + cix:2 + cix])
                cls01 = pt.tile([128, 1], F32, name=f"cls01_{i}", tag="cls01")
                V.tensor_tensor(out=cls01[:, :], in0=clsc[:, 0:1], in1=clsc[:, 1:2],
                                op=OP.subtract)
                V.tensor_tensor(out=cls01[:, :], in0=cls01[:, :], in1=clsc[:, 2:3],
                                op=OP.subtract)
                V.tensor_tensor(out=fin6[:, i:i + 1], in0=cls01[:, :],
                                in1=clsc[:, 3:4], op=OP.subtract)

                # loc
                lsum = ps.tile([H, AW], F32, name=f"lsum{i}", tag="spF")
                for dn, (nm, inv, off, pp_) in enumerate((
                        ("x", invaw, True, pv[:, :, 0, :]),
                        ("y", invah, True, pv[:, :, 1, :]),
                        ("w", logaw, None, pv[:, :, 2, :]),
                        ("h", logah, None, pv[:, :, 3, :]))):
                    pair_src, off16 = {"x": ("xy", 1), "y": ("xy", 0),
                                       "w": ("wh", 1), "h": ("wh", 0)}[nm]
                    acc16 = accs[pair_src].bitcast(F16)
                    accf = ps.tile([H, AW], F32, name=f"af{nm}{i}", tag="spA")
                    S.activation(out=accf[:, :],
                                 in_=acc16.rearrange("p (aw two) -> p aw two", two=2)
                                 [:, :, off16], func=AF.Copy)
                    d = ps.tile([H, AW], F32, name=f"d{nm}{i}", tag="spC")
                    if off is not None:
                        # host pre-added axw/ayh to pred: d = pred' - acc*inv
                        gxw = ps.tile([H, AW], F32, name=f"gxw{nm}{i}", tag="spD")
                        V.tensor_tensor(out=gxw[:, :], in0=accf[:, :], in1=inv[:, :],
                                        op=OP.mult)
                        V.tensor_tensor(out=d[:, :], in0=pp_, in1=gxw[:, :],
                                        op=OP.subtract)
                    else:
                        # host pre-added logaw/logah to pred: d = pred' - acc
                        V.tensor_tensor(out=d[:, :], in0=pp_, in1=accf[:, :],
                                        op=OP.subtract)
                    # smoothl1(d) = 0.5*d^2 - 0.5*relu(|d|-1)^2  (exact)
                    absd = ps.tile([H, AW], F32, name=f"ab{nm}{i}", tag="sAbs")
                    S.activation(out=absd[:, :], in_=d[:, :], func=AF.Abs)
                    sqd = ps.tile([H, AW], F32, name=f"sqd{nm}{i}", tag="sSq")
                    S.activation(out=sqd[:, :], in_=d[:, :], func=AF.Square,
                                 scale=0.7071067811865476)
                    rll = ps.tile([H, AW], F32, name=f"rl{nm}{i}", tag="spD")
                    S.activation(out=rll[:, :], in_=absd[:, :], func=AF.Relu,
                                 bias=negone[:, :])
                    rsq = ps.tile([H, AW], F32, name=f"rs{nm}{i}", tag="spE")
                    S.activation(out=rsq[:, :], in_=rll[:, :], func=AF.Square,
                                 scale=0.7071067811865476)
                    cc_ = ps.tile([H, AW], F32, name=f"cc{nm}{i}", tag="spD")
                    V.tensor_tensor(out=cc_[:, :], in0=sqd[:, :], in1=rsq[:, :],
                                    op=OP.subtract)
                    if dn == 0:
                        V.tensor_copy(out=lsum[:, :], in_=cc_[:, :])
                    elif dn < 3:
                        V.tensor_tensor(out=lsum[:, :], in0=lsum[:, :], in1=cc_[:, :],
                                        op=OP.add)
                    else:
                        fin = ps.tile([H, AW], F32, name=f"finL{i}", tag="spE")
                        V.tensor_tensor(out=fin[:, :], in0=lsum[:, :], in1=cc_[:, :],
                                        op=OP.add)
                        V.scalar_tensor_tensor(out=fin[:, :], in0=fin[:, :],
                                               scalar=1.0, in1=pos_f[:, :],
                                               op0=OP.mult, op1=OP.mult,
                                               accum_out=fin6[:, 2 + i:3 + i])

            if STOP_STAGE < 5:
                early_out()
                nc.compile()
                return nc

            # ============ SRCH: merged binary search (fp16 counts) ============
            thb = pe.tile([128, 2], F32, name="thb")
            V.memset(thb[:, :], 0.0)
            cnt2 = pe.tile([128, 2], F32, name="cnt2")
            cnt2a = pe.tile([128, 2], F32, name="cnt2a")
            j16 = ps.tile([H, AW], F16, name="junk16", tag="junk16")
            junk16 = [j16 for _ in range(n_img)]
            for it in range(N_SEARCH):
                s_i = SEARCH_SPAN * (0.5 ** it)
                for i in range(n_img):
                    V.tensor_scalar(out=junk16[i][:, :], in0=st[i]["zt"][:, :],
                                    scalar1=thb[:, i:i + 1], scalar2=None,
                                    op0=OP.is_gt, op1=OP.add,
                                    accum_out=cnt2[:, i:i + 1])
                G.partition_all_reduce(cnt2a[:, :], cnt2[:, :], channels=128,
                                       reduce_op=RED.add)
                ge = pt.tile([128, 2], F32, name=f"ge{it}", tag="ge")
                V.tensor_tensor(out=ge[:, :], in0=cnt2a[:, :], in1=kk[:, :], op=OP.is_ge)
                V.scalar_tensor_tensor(out=thb[:, :], in0=ge[:, :], scalar=2.0 * s_i,
                                       in1=thb[:, :], op0=OP.mult, op1=OP.add)
                V.tensor_single_scalar(out=thb[:, :], in_=thb[:, :], scalar=s_i,
                                       op=OP.subtract)

            # final count + masked softplus sum: [cntF0, cntF1, sel0, sel1]
            cntF = pe.tile([128, 4], F32, name="cntF")
            for i in range(n_img):
                V.tensor_scalar(out=junk16[i][:, :], in0=st[i]["zt"][:, :],
                                scalar1=thb[:, i:i + 1], scalar2=None,
                                op0=OP.is_gt, op1=OP.add,
                                accum_out=cntF[:, i:i + 1])
                V.scalar_tensor_tensor(out=junk16[i][:, :], in0=st[i]["zt"][:, :],
                                       scalar=thb[:, i:i + 1], in1=st[i]["spz"][:, :],
                                       op0=OP.is_gt, op1=OP.mult,
                                       accum_out=cntF[:, 2 + i:3 + i])
            cntFa = pe.tile([128, 4], F32, name="cntFa")
            G.partition_all_reduce(cntFa[:, :], cntF[:, :], channels=128,
                                   reduce_op=RED.add)
            tha = pt.tile([128, 2], F32, name="tha", tag="tha")
            S.activation(out=tha[:, :], in_=thb[:, :], func=AF.Abs)
            the = pt.tile([128, 2], F32, name="the", tag="the")
            S.activation(out=the[:, :], in_=tha[:, :], func=AF.Exp, scale=-1.0)
            thl = pt.tile([128, 2], F32, name="thl", tag="thl")
            S.activation(out=thl[:, :], in_=the[:, :], func=AF.Ln, bias=1.0)
            thr = pt.tile([128, 2], F32, name="thr", tag="thr")
            S.activation(out=thr[:, :], in_=thb[:, :], func=AF.Relu)
            sth = pt.tile([128, 2], F32, name="sth", tag="sth")
            V.tensor_tensor(out=sth[:, :], in0=thr[:, :], in1=thl[:, :], op=OP.add)
            kc = pt.tile([128, 2], F32, name="kc", tag="kc")
            V.tensor_tensor(out=kc[:, :], in0=kk[:, :], in1=cntFa[:, 0:2], op=OP.subtract)
            kcs = pt.tile([128, 2], F32, name="kcs", tag="kcs")
            V.tensor_tensor(out=kcs[:, :], in0=kc[:, :], in1=sth[:, :], op=OP.mult)
            objneg = pt.tile([128, 2], F32, name="objneg", tag="objneg")
            V.tensor_tensor(out=objneg[:, :], in0=kcs[:, :], in1=cntFa[:, 2:4], op=OP.add)

            # ============ FIN ============
            fin6a = pe.tile([128, 6], F32, name="fin6a")
            G.partition_all_reduce(fin6a[:, :], fin6[:, :], channels=128,
                                   reduce_op=RED.add)
            res_t = pt.tile([1, 8], F32, name="res_t", tag="res_t")
            V.memset(res_t[:, :], 0.0)
            ob2 = pt.tile([1, 2], F32, name="ob2", tag="ob2")
            V.tensor_tensor(out=ob2[:, :], in0=fin6a[0:1, 4:6], in1=objneg[0:1, :],
                            op=OP.add)
            V.tensor_reduce(out=res_t[:1, 0:1], in_=ob2[:1, :], axis=AX.X, op=OP.add)
            V.tensor_reduce(out=res_t[:1, 1:2], in_=fin6a[0:1, 0:2], axis=AX.X, op=OP.add)
            V.tensor_reduce(out=res_t[:1, 2:3], in_=fin6a[0:1, 2:4], axis=AX.X, op=OP.add)
            V.tensor_reduce(out=res_t[:1, 3:4], in_=cnt4a[0:1, 0:2], axis=AX.X, op=OP.add)
            ns2 = pt.tile([1, 2], F32, name="ns2", tag="ns2")
            V.tensor_tensor(out=ns2[:, :], in0=cnt4a[0:1, 0:2], in1=kk[0:1, :], op=OP.add)
            V.tensor_reduce(out=res_t[:1, 4:5], in_=ns2[:1, :], axis=AX.X, op=OP.add)
            nc.sync.dma_start(out=d_res[:, :], in_=res_t[:, :])

    nc.compile()
    return nc


def prep_inputs(pred, anchors, gt_boxes, gt_labels, n_img=N_IMG):
    pred = np.ascontiguousarray(pred, dtype=np.float32)
    anchors = np.asarray(anchors, dtype=np.float32)
    gt_boxes = np.asarray(gt_boxes, dtype=np.float32)
    gt_labels = np.asarray(gt_labels)

    anc = anchors.reshape(H, W, A, 4)
    x1 = anc[0, :, :, 0]  # [W, A]
    x2 = anc[0, :, :, 2]
    y1 = anc[:, 0, :, 1]  # [H, A]
    y2 = anc[:, 0, :, 3]
    areaA = ((anc[0, 0, :, 2] - anc[0, 0, :, 0])
             * (anc[0, 0, :, 3] - anc[0, 0, :, 1])).astype(np.float32)  # [A]

    anc_pl = anchors.reshape(H, W, A, 4).transpose(3, 0, 2, 1).reshape(4, H, A * W)
    ax1, ay1, ax2, ay2 = anc_pl
    awr = np.maximum(ax2 - ax1, 1e-6)
    ahr = np.maximum(ay2 - ay1, 1e-6)
    invaw = (1.0 / awr).astype(np.float32)
    invah = (1.0 / ahr).astype(np.float32)
    axw = ((ax1 + ax2) * 0.5 * invaw).astype(np.float32)
    ayh = ((ay1 + ay2) * 0.5 * invah).astype(np.float32)
    logaw = np.log(awr).astype(np.float32)
    logah = np.log(ahr).astype(np.float32)
    locp = np.stack([invaw, invah, axw, ayh, logaw, logah])

    ipat = np.ascontiguousarray(
        np.broadcast_to(((31 - np.arange(M, dtype=np.uint32)) << 2), (H, M)))

    in_maps = []
    n_cores = pred.shape[0] // n_img
    for cix in range(n_cores):
        sl = slice(cix * n_img, (cix + 1) * n_img)
        gtb_c = gt_boxes[sl]  # [n_img, M, 4]

        # pair-stage tables (all f32 math, mirrors the reference chain)
        gx1 = gtb_c[:, :, 0]; gy1 = gtb_c[:, :, 1]
        gx2 = gtb_c[:, :, 2]; gy2 = gtb_c[:, :, 3]
        areaB = ((gx2 - gx1) * (gy2 - gy1)).astype(np.float32)      # [n,M]
        c_am = (areaA[None, None, :] + areaB[:, :, None]).astype(np.float32)  # [n,M,A]
        invc = (np.float32(1.0) / c_am).astype(np.float32)
        # wx[n, M, A, W]
        wx = np.minimum(x2[None, None], gx2[:, :, None, None]) \
            - np.maximum(x1[None, None], gx1[:, :, None, None])
        wx = np.maximum(wx.transpose(0, 1, 3, 2), 0.0).astype(np.float32)  # [n,M,A,W]
        wx = (wx * invc[:, :, :, None]).astype(np.float32)
        wy = np.minimum(y2[None, None], gy2[:, :, None, None]) \
            - np.maximum(y1[None, None], gy1[:, :, None, None])
        wy = np.maximum(wy.transpose(0, 1, 3, 2), 0.0).astype(np.float32)  # [n,M,A,H]

        bd = np.zeros((n_img, A, 2, 16, 2048), np.float16)
        wyt = np.zeros((n_img, A, 2, 16, 128), np.float16)
        for hf in range(2):
            for r in range(16):
                m = hf * 16 + r
                bd[:, :, hf, r, r * 128:(r + 1) * 128] = wx[:, m].transpose(0, 1, 2)
                wyt[:, :, hf, r, :] = wy[:, m]

        gtl_r = np.ascontiguousarray(np.broadcast_to(
            gt_labels[sl].astype(np.float32)[:, None, :], (n_img, H, M)))
        gcx_v = ((gtb_c[:, :, 0] + gtb_c[:, :, 2]) * 0.5).astype(np.float32)
        gcy_v = ((gtb_c[:, :, 1] + gtb_c[:, :, 3]) * 0.5).astype(np.float32)
        lgw_v = np.log(np.maximum(gtb_c[:, :, 2] - gtb_c[:, :, 0], 1e-6)).astype(np.float32)
        lgh_v = np.log(np.maximum(gtb_c[:, :, 3] - gtb_c[:, :, 1], 1e-6)).astype(np.float32)

        def pack16(hi, lo):
            hb = hi.astype(np.float16).view(np.uint16).astype(np.uint32)
            lb = lo.astype(np.float16).view(np.uint16).astype(np.uint32)
            return (hb << np.uint32(16)) | lb
        pb_v = np.zeros((n_img, H, 2 * M), np.uint32)
        pb_v[:, :, 0::2] = pack16(gcx_v, gcy_v)[:, None, :]
        pb_v[:, :, 1::2] = pack16(lgw_v, lgh_v)[:, None, :]
        pred_c = np.array(pred[sl])
        # fold anchor offsets into pred box channels: layout [n, A*K, H, W]
        axw_w = axw.reshape(H, A, W)[0]      # [A, W] (x offsets, h-invariant)
        ayh_h = ayh.reshape(H, A, W)[:, :, 0]  # [H, A] (y offsets, w-invariant)
        lgw_aw = logaw.reshape(H, A, W)[0]   # [A, W]
        lgh_aw = logah.reshape(H, A, W)[0]
        for a in range(A):
            pred_c[:, a * K + 0] += axw_w[a][None, None, :]
            pred_c[:, a * K + 1] += ayh_h[:, a][None, :, None]
            pred_c[:, a * K + 2] += lgw_aw[a][None, None, :]
            pred_c[:, a * K + 3] += lgh_aw[a][None, None, :]
        in_maps.append({
            "pred": np.ascontiguousarray(pred_c),
            "bd": bd,
            "wyt": wyt,
            "gtl": gtl_r,
            "ipat": ipat,
            "pb": pb_v,
            "locp": locp,
        })
    return in_maps


def finalize(partials):
    tot = np.sum(np.stack([np.asarray(p).reshape(8) for p in partials]),
                 axis=0, dtype=np.float64)
    obj_s, cls_s, loc_s, total_pos, total_sel = tot[:5]
    obj_s, cls_s, loc_s = np.float32(obj_s), np.float32(cls_s), np.float32(loc_s)
    denom_pos = np.float32(max(total_pos, 1.0))
    denom_obj = np.float32(max(total_sel, 1.0))
    loss_loc = np.float32(loc_s / denom_pos)
    loss_cls = np.float32(cls_s / denom_pos)
    loss_obj = np.float32(obj_s / denom_obj)
    loss_total = np.float32(2.0 * loss_loc + 1.0 * loss_cls + 1.0 * loss_obj)
    return np.array([loss_obj, loss_cls, loss_loc, loss_total], dtype=np.float32)


_NC_CACHE = {}


def _get_nc():
    if "nc" not in _NC_CACHE:
        _NC_CACHE["nc"] = build_nc()
    return _NC_CACHE["nc"]


def run_with_results(pred, anchors, gt_boxes, gt_labels, trace=False, **kw):
    nc = _get_nc()
    in_maps = prep_inputs(pred, anchors, gt_boxes, gt_labels)
    res = run_bass_kernel_spmd(nc, in_maps, list(range(N_CORES)), trace=trace, **kw)
    out = finalize([res.results[c]["res"] for c in range(N_CORES)])
    return out, res


def kernel(pred, anchors, gt_boxes, gt_labels):
    return run_with_results(pred, anchors, gt_boxes, gt_labels)[0]
